# revision 34
# baseline (speedup 1.0000x reference)
"""Trainium2 Bass kernel for MADE autoregressive sampling (rsample).

Structure (degrees mh = arange(512)%63 + 1, sorted into 4 groups of 128):
  - per step i (degree i), the m<=9 units of that degree are computed once;
  - unit-land layout: features on partitions, batch on free dim, column
    j = c*128 + b  <->  batch row (within the 512-col half) c*128 + b
    (chunk-major, c = batch chunk of 128);
  - batch-land: output accumulators outp [128 batch, 4ch x 128 outs].

Critical-path design (vs. the DMA-scatter baseline):
  - z_i is computed batch-land (exp/ln softplus + mult/add on [128, 2]
    chunk-pairs), written into zBb columns, then PE-TRANSPOSED (32-dim
    block x 4 chunks) into PSUM and copied back to the unit-land zTbx
    rows with 32-aligned partition bases -- no per-step DMA round trip
    on the recurrence;
  - the h2 cross-group prefix is read directly from h2preS[g][0:r0+m]
    (partition base 0) via a selector-identity matmul (L2s) accumulating
    into the same PSUM group as L2p/L2d -- no h2-stage DMA;
  - the two batch halves (waves) are software-pipelined: each emission
    segment holds wave w's front (L1/relus/L2/L3) interleaved with the
    other wave's back (extract/transpose/copy), so cross-engine round
    trips of one wave overlap compute of the other;
  - L1 is split into two 256-col matmuls so each copy lane (chunks 0,1 /
    2,3) unblocks its relu1 half independently;
  - ctx projection still uses the staged identity-rows trick in w1x
    (rows 64..81 of zTbx, gpsimd DMA, 2-step lookahead, off-path).
"""

import ml_dtypes
import numpy as np

B, D, CTX, H = 8192, 64, 256, 512
NCORES = 8
BS = B // NCORES   # 1024 rows per core
NH = BS // 2       # 512 per half/wave
MMAX = 9           # max units per degree
KZ = D + 2 * MMAX  # 82: z rows + two ctx-stage slots

BF = ml_dtypes.bfloat16


def _structure():
    m0 = np.arange(1, D + 1)
    mh = (np.arange(H) % (D - 1)) + 1
    M1 = (mh[:, None] >= m0[None, :]).astype(np.float32)   # (H, D)
    M2 = (mh[:, None] >= mh[None, :]).astype(np.float32)   # (H, H)
    mo = np.concatenate([m0, m0])
    Mo = (mo[:, None] > mh[None, :]).astype(np.float32)    # (2D, H)
    perm = np.argsort(mh, kind="stable")
    smh = mh[perm]
    S = np.zeros(D, np.int64)
    E = np.zeros(D, np.int64)
    for i in range(1, D):
        S[i] = np.searchsorted(smh, i, side="left")
        E[i] = np.searchsorted(smh, i, side="right")
    return M1, M2, Mo, perm, S, E


_M1, _M2, _Mo, _PERM, _S, _E = _structure()
assert int(_E[15]) == 128 and int(_E[31]) == 256 and int(_E[47]) == 384

# push schedule: step -> list of (source group G, target group g2).  The
# urgent G->G+1 pushes run at the first step of the target group (the
# boundary step's h1g DMA has landed by then); farther targets spread out.
_PUSHES = {16: [(0, 1)], 18: [(0, 2)], 20: [(0, 3)],
           32: [(1, 2)], 34: [(1, 3)], 48: [(2, 3)]}
# steps whose h2 prefix is read via selector matmul (the group's push
# lands the same step, too late for the 2-step-lookahead stage DMA)
L2SEL_STEPS = {16, 17, 32, 33, 48, 49}


def _host_weights(W1, b1, Wc, W2, b2, Wo, bo):
    W1m = (W1 * _M1).T[:, _PERM]                     # (64, 512)
    W2m = ((W2 * _M2).T)[_PERM][:, _PERM]            # (512, 512)
    Wom = ((Wo * _Mo).T)[_PERM, :]                   # (512, 128)
    Wcs = Wc[_PERM]                                  # (512, 256)
    b1s_ = b1[_PERM]
    b2s_ = b2[_PERM]

    # stacked L1 weights [W1m slice; I@slot(i%2)] and L2 diag+stage
    # [W2 diag; I] (identity rows add the staged h2 cross-group prefix
    # during the same PE pass)
    w1x = np.zeros((KZ, D * MMAX), np.float32)
    w2dx = np.zeros((2 * MMAX, D * MMAX), np.float32)
    selx = np.zeros((128, D * MMAX), np.float32)
    for i in sorted(L2SEL_STEPS):
        s, e = int(_S[i]), int(_E[i])
        r0, m = s % 128, e - s
        for k in range(m):
            selx[r0 + k, i * MMAX + k] = 1.0
    womp = np.zeros((MMAX, D * 2 * D), np.float32)
    b2p = np.zeros((MMAX, D), np.float32)
    for i in range(1, D):
        s, e = int(_S[i]), int(_E[i])
        m = e - s
        w1x[0:D, i * MMAX:i * MMAX + m] = W1m[:, s:e]
        slot = D + MMAX * (i % 2)
        for k in range(m):
            w1x[slot + k, i * MMAX + k] = 1.0
        w2dx[:m, i * MMAX:i * MMAX + m] = W2m[s:e, s:e]
        if i >= 16 and i not in L2SEL_STEPS:
            for k in range(m):
                w2dx[m + k, i * MMAX + k] = 1.0
        womp[:m, i * 2 * D:(i + 1) * 2 * D] = Wom[s:e, :]
        b2p[:m, i] = b2s_[s:e]

    return {
        "w1x": w1x.astype(BF),
        "w2m": np.ascontiguousarray(W2m).astype(BF),
        "wct": np.ascontiguousarray(Wcs.T).astype(BF),        # (256, 512)
        "w2dx": w2dx.astype(BF),
        "selx": selx.astype(BF),
        "womp": womp.astype(BF),
        "b1s": np.ascontiguousarray(b1s_.reshape(4, 128).T, np.float32),
        "b2p": b2p,
        "b2g": np.ascontiguousarray(b2s_.reshape(4, 128).T, np.float32),
        "bo4": np.ascontiguousarray(np.tile(bo, 4)[None, :]).astype(BF),
        "ones": np.ones((1, 128), np.float32).astype(BF),
        "ident": np.eye(128, dtype=np.float32).astype(BF),
    }


_NC_CACHE = {}


def _build():
    if "nc" in _NC_CACHE:
        return _NC_CACHE["nc"]
    from contextlib import ExitStack

    import concourse.mybir as mybir
    import concourse.tile as tile
    from concourse import bacc

    f32 = mybir.dt.float32
    bf16 = mybir.dt.bfloat16
    AF = mybir.ActivationFunctionType
    OP = mybir.AluOpType

    # Only Exp/Ln/Relu/Identity are used -- pin the one ACT table so the
    # greedy selection pass never inserts mid-kernel table loads.
    import concourse.bacc as bacc_mod
    _orig_tables = bacc_mod.get_activation_tables

    def _one_table(arch):
        tabs = _orig_tables(arch)
        return {k: (v if k == "natural_log_exp_and_others" else set())
                for k, v in tabs.items()}

    bacc_mod.get_activation_tables = _one_table

    nc = bacc.Bacc("TRN2", target_bir_lowering=False)

    ctxT_d = nc.dram_tensor("ctxT", [CTX, BS], bf16, kind="ExternalInput")
    epsB_d = nc.dram_tensor("epsB", [2, 128, 4 * D], f32, kind="ExternalInput")
    w1x_d = nc.dram_tensor("w1x", [KZ, D * MMAX], bf16, kind="ExternalInput")
    w2m_d = nc.dram_tensor("w2m", [H, H], bf16, kind="ExternalInput")
    wct_d = nc.dram_tensor("wct", [CTX, H], bf16, kind="ExternalInput")
    w2dx_d = nc.dram_tensor("w2dx", [2 * MMAX, D * MMAX], bf16,
                            kind="ExternalInput")
    selx_d = nc.dram_tensor("selx", [128, D * MMAX], bf16,
                            kind="ExternalInput")
    womp_d = nc.dram_tensor("womp", [MMAX, D * 2 * D], bf16,
                            kind="ExternalInput")
    b1s_d = nc.dram_tensor("b1s", [128, 4], f32, kind="ExternalInput")
    b2p_d = nc.dram_tensor("b2p", [MMAX, D], f32, kind="ExternalInput")
    b2g_d = nc.dram_tensor("b2g", [128, 4], f32, kind="ExternalInput")
    bo4_d = nc.dram_tensor("bo4", [1, 4 * 2 * D], bf16, kind="ExternalInput")
    ones_d = nc.dram_tensor("ones", [1, 128], bf16, kind="ExternalInput")
    ident_d = nc.dram_tensor("ident", [128, 128], bf16, kind="ExternalInput")

    # outputs, batch-major (BS, D); rows r = h*512 + c*128 + p
    zo_d = nc.dram_tensor("zo", [BS, D], f32, kind="ExternalOutput")
    mo_d = nc.dram_tensor("mo", [BS, D], f32, kind="ExternalOutput")
    so_d = nc.dram_tensor("so", [BS, D], f32, kind="ExternalOutput")

    NQ = NH // 2  # 256: column half of a wave (chunk pair)

    with tile.TileContext(nc) as tc, ExitStack() as ctx:
        const = ctx.enter_context(tc.tile_pool(name="const", bufs=1))
        work = ctx.enter_context(tc.tile_pool(name="work", bufs=6))
        pout = ctx.enter_context(tc.tile_pool(name="pout", bufs=1, space="PSUM"))
        pscr = ctx.enter_context(tc.tile_pool(name="pscr", bufs=1, space="PSUM"))

        # ---- constant loads (spread across the 3 DMA queues) ----
        wct = [const.tile([128, H], bf16, name=f"wct{k}") for k in range(2)]
        ctxT = [const.tile([128, BS], bf16, name=f"ctxTs{k}") for k in range(2)]
        for k in range(2):
            nc.sync.dma_start(wct[k][:, :], wct_d[k * 128:(k + 1) * 128, :])
            nc.scalar.dma_start(ctxT[k][:, :],
                                ctxT_d[k * 128:(k + 1) * 128, :])
        w1x = const.tile([KZ, D * MMAX], bf16)
        nc.sync.dma_start(w1x[:, :], w1x_d[:, :])
        w2dx = const.tile([2 * MMAX, D * MMAX], bf16)
        nc.scalar.dma_start(w2dx[:, :], w2dx_d[:, :])
        selx = const.tile([128, D * MMAX], bf16)
        nc.sync.dma_start(selx[:, :], selx_d[:, :])
        womp = const.tile([MMAX, D * 2 * D], bf16)
        nc.scalar.dma_start(womp[:, :], womp_d[:, :])
        b1s = const.tile([128, 4], f32)
        nc.sync.dma_start(b1s[:, :], b1s_d[:, :])
        b2p = const.tile([MMAX, D], f32)
        nc.scalar.dma_start(b2p[:, :], b2p_d[:, :])
        b2g = const.tile([128, 4], f32)
        nc.sync.dma_start(b2g[:, :], b2g_d[:, :])
        bo4 = const.tile([1, 4 * 2 * D], bf16)
        nc.sync.dma_start(bo4[:, :], bo4_d[:, :])
        ones = const.tile([1, 128], bf16)
        nc.scalar.dma_start(ones[:, :], ones_d[:, :])
        ident = const.tile([128, 128], bf16)
        nc.sync.dma_start(ident[:, :], ident_d[:, :])
        epsB = [const.tile([128, 4 * D], f32, name=f"epsB{h}") for h in range(2)]
        for h in range(2):
            nc.scalar.dma_start(epsB[h][:, :], epsB_d[h, :, :])
        w2m = [const.tile([128, H], bf16, name=f"w2m{g}") for g in range(4)]
        for g in range(4):
            (nc.sync if g % 2 else nc.scalar).dma_start(
                w2m[g][:, :], w2m_d[g * 128:(g + 1) * 128, :])

        # ---- state tiles ----
        zTbx = const.tile([KZ, BS], bf16)      # unit-land z + ctx slots
        nc.vector.memset(zTbx[:, :], 0.0)
        zBb = [const.tile([128, 4 * D], bf16, name=f"zBb{h}") for h in range(2)]
        for h in range(2):
            nc.gpsimd.memset(zBb[h][:, :], 0.0)
        h1g = [const.tile([128, BS], bf16, name=f"h1g{g}") for g in range(4)]
        cbg = [const.tile([128, BS], bf16, name=f"cbg{g}") for g in range(4)]
        h2preS = {g: const.tile([128, BS], bf16, name=f"h2preS{g}")
                  for g in (1, 2, 3)}
        h2preF = {g: const.tile([128, BS], f32, name=f"h2preF{g}")
                  for g in (2, 3)}
        muB = [const.tile([128, 4 * D], f32, name=f"muB{h}") for h in range(2)]
        scB = [const.tile([128, 4 * D], f32, name=f"scB{h}") for h in range(2)]
        zB = [const.tile([128, 4 * D], f32, name=f"zB{h}") for h in range(2)]
        zt2 = [const.tile([128, 4 * D], f32, name=f"zt2{h}") for h in range(2)]

        # persistent PSUM: output accumulators + transposed-z staging
        outp = [pout.tile([128, 4 * 128], f32, name=f"outp{h}", tag=f"op{h}")
                for h in range(2)]
        zT = pout.tile([D, BS], bf16, name="zT", tag="zT")
        fil = pout.tile([1, NH], f32, name="fil", tag="fil")

        def ov(h, ch):            # (128, 128) chunk view of the accumulator
            return outp[h][:, ch * 128:(ch + 1) * 128]

        def ocol(h, o):           # (128, 4) strided column view, output o
            return outp[h][:, :].rearrange("p (c o) -> p c o", c=4)[:, :, o]

        def bcol(t, i):           # (128, 4)
            return t[:, :].rearrange("p (c d) -> p c d", c=4)[:, :, i]

        # ---- init: bias rows (one f=512 matmul per half) ----
        for h in range(2):
            nc.tensor.matmul(outp[h][:, :], ones[0:1, :], bo4[0:1, :],
                             start=True, stop=True)

        # ---- init: ctx projection (+b1), unit-land, bf16 out ----
        for g in range(4):
            for h in range(2):
                cs = slice(h * NH, (h + 1) * NH)
                pc = pscr.tile([128, NH], f32, tag="pp", bufs=2,
                               name=f"pc{g}_{h}")
                for k in range(2):
                    nc.tensor.matmul(pc[:, :], wct[k][:, g * 128:(g + 1) * 128],
                                     ctxT[k][:, cs], start=(k == 0),
                                     stop=(k == 1))
                eng = nc.scalar if (g + h) % 2 == 0 else nc.vector
                if eng is nc.scalar:
                    eng.activation(cbg[g][:, cs], pc[:, :], AF.Identity,
                                   bias=b1s[:, g:g + 1])
                else:
                    eng.tensor_scalar(cbg[g][:, cs], pc[:, :], b1s[:, g:g + 1],
                                      0.0, OP.add, OP.add)

        # ---- stage helper: ctx rows for step i into zTbx slot i%2 ----
        def emit_cstage(i):
            if i >= D:
                return
            s, e = int(_S[i]), int(_E[i])
            g, r0, m = s // 128, s % 128, e - s
            slot = D + MMAX * (i % 2)
            nc.gpsimd.dma_start(zTbx[slot:slot + m, :], cbg[g][r0:r0 + m, :])

        hx = {}

        def alloc_hx(i):
            if i < D:
                hx[i] = work.tile([2 * MMAX, BS], bf16, tag="hx",
                                  name=f"hx{i}")

        def emit_h2stage(i):
            s, e = int(_S[i]), int(_E[i])
            g, r0, m = s // 128, s % 128, e - s
            nc.gpsimd.dma_start(hx[i][m:2 * m, :], h2preS[g][r0:r0 + m, :])

        # p1 lives at partitions 0:9 and p2 at 32:41 of one shared PSUM
        # bank per wave (32-aligned bases are legal compute APs).
        p12t = {}

        def front1(w, i):
            """L1 into p1 (rows 0:9 of the shared bank), 256-col lanes."""
            s, e = int(_S[i]), int(_E[i])
            m = e - s
            p12 = pscr.tile([32 + MMAX, NH], f32, tag=f"p12_{w}",
                            name=f"p12_{i}_{w}")
            p12t[w] = p12
            for q in range(2):
                csq = slice(w * NH + q * NQ, w * NH + (q + 1) * NQ)
                nc.tensor.matmul(p12[0:m, q * NQ:(q + 1) * NQ],
                                 w1x[:, i * MMAX:i * MMAX + m],
                                 zTbx[:, csq], start=True, stop=True)

        def front2(w, i):
            """L2 prefix matmul (within-group, independent of z_i)."""
            s, e = int(_S[i]), int(_E[i])
            g, r0, m = s // 128, s % 128, e - s
            if r0 > 0:
                cs = slice(w * NH, (w + 1) * NH)
                nc.tensor.matmul(p12t[w][32:32 + m, :], w2m[g][0:r0, s:e],
                                 h1g[g][0:r0, cs], start=True, stop=False)

        def front_relu1(w, i):
            s, e = int(_S[i]), int(_E[i])
            m = e - s
            p1 = p12t[w]
            ca = slice(w * NH, w * NH + NQ)
            cb = slice(w * NH + NQ, (w + 1) * NH)
            nc.scalar.activation(hx[i][0:m, ca], p1[0:m, 0:NQ], AF.Relu)
            nc.vector.tensor_relu(hx[i][0:m, cb], p1[0:m, NQ:NH])

        def front_h1g(w, i):
            if i >= D - 1:
                return    # no consumer for the last step's h1
            s, e = int(_S[i]), int(_E[i])
            g, r0, m = s // 128, s % 128, e - s
            cs = slice(w * NH, (w + 1) * NH)
            nc.sync.dma_start(h1g[g][r0:r0 + m, cs], hx[i][0:m, cs])

        def front3(w, i):
            """L2s (h2 cross-group prefix via selector) + L2d (diag)."""
            s, e = int(_S[i]), int(_E[i])
            g, r0, m = s // 128, s % 128, e - s
            p2 = p12t[w][32:32 + MMAX, :]
            if r0 <= 64:
                # small/no L2p: bridge the relu1 wait so the PE p-state
                # ramp survives (fillers are always-ready)
                for _ in range(2):
                    nc.tensor.matmul(fil[0:1, 0:NQ], ones[0:1, 0:1],
                                     w1x[0:1, 0:NQ], start=True, stop=True)
            if i in L2SEL_STEPS:
                k2 = m
            else:
                k2 = 2 * m if g >= 1 else m
            for q in range(2):
                csq = slice(w * NH + q * NQ, w * NH + (q + 1) * NQ)
                qs = slice(q * NQ, (q + 1) * NQ)
                started = r0 > 0
                if i in L2SEL_STEPS:
                    nc.tensor.matmul(p2[0:m, qs],
                                     selx[0:r0 + m, i * MMAX:i * MMAX + m],
                                     h2preS[g][0:r0 + m, csq],
                                     start=not started, stop=False)
                    started = True
                nc.tensor.matmul(p2[0:m, qs],
                                 w2dx[0:k2, i * MMAX:i * MMAX + m],
                                 hx[i][0:k2, csq], start=not started, stop=True)

        h2nt = {}

        def front_relu2(w, i):
            s, e = int(_S[i]), int(_E[i])
            g, m = s // 128, e - s
            p2 = p12t[w][32:32 + MMAX, :]
            t = work.tile([MMAX, NH], bf16, tag=f"h2n{w}", name=f"h2n_{i}_{w}")
            h2nt[w] = t
            if g == 0:
                nc.scalar.activation(t[0:m, 0:NQ], p2[0:m, 0:NQ], AF.Relu,
                                     bias=b2p[0:m, i:i + 1])
                nc.vector.tensor_scalar(t[0:m, NQ:NH], p2[0:m, NQ:NH],
                                        b2p[0:m, i:i + 1], 0.0,
                                        OP.add, OP.max)
            else:
                nc.scalar.activation(t[0:m, 0:NQ], p2[0:m, 0:NQ], AF.Relu)
                nc.vector.tensor_relu(t[0:m, NQ:NH], p2[0:m, NQ:NH])

        def back_l3(w, i):
            s, e = int(_S[i]), int(_E[i])
            m = e - s
            t = h2nt[w]
            for ch in range(4):
                nc.tensor.matmul(ov(w, ch), t[0:m, ch * 128:(ch + 1) * 128],
                                 womp[0:m, i * 2 * D:(i + 1) * 2 * D],
                                 start=False, stop=True)

        def back_extract(w, i):
            """softplus + z_i, batch-land [128, 4]."""
            nc.scalar.activation(bcol(scB[w], i), ocol(w, D + i), AF.Exp)
            nc.scalar.activation(bcol(scB[w], i), bcol(scB[w], i),
                                 AF.Ln, bias=1.0)
            if i >= D - 1:
                return
            zt = work.tile([128, 4], f32, tag=f"zt{w}", name=f"zt{i}_{w}")
            nc.vector.tensor_tensor(zt[:, :], bcol(scB[w], i),
                                    bcol(epsB[w], i), OP.mult)
            nc.vector.tensor_tensor(bcol(zBb[w], i), zt[:, :],
                                    ocol(w, i), OP.add)

        def back_transpose(w, i):
            if i >= D - 1:
                return
            b32 = (i // 32) * 32
            for c in range(4):
                nc.tensor.transpose(
                    zT[b32:b32 + 32, w * NH + c * 128:w * NH + (c + 1) * 128],
                    zBb[w][:, c * D + b32:c * D + b32 + 32],
                    ident[:, :])

        def back_copy(w, i):
            if i >= D - 1:
                return
            b32 = (i // 32) * 32
            cs0 = slice(w * NH, w * NH + NQ)
            cs1 = slice(w * NH + NQ, (w + 1) * NH)
            nc.scalar.activation(zTbx[b32:b32 + 32, cs0],
                                 zT[b32:b32 + 32, cs0], AF.Identity)
            nc.vector.tensor_scalar(zTbx[b32:b32 + 32, cs1],
                                    zT[b32:b32 + 32, cs1], 0.0, 0.0,
                                    OP.add, OP.add)

        def emit_pushes(i):
            for G, g2 in _PUSHES.get(i, ()):
                for h in range(2):
                    cs = slice(h * NH, (h + 1) * NH)
                    pp = pscr.tile([128, NH], f32, tag="pp", bufs=2,
                                   name=f"push{G}_{g2}_{h}")
                    nc.tensor.matmul(pp[:, :],
                                     w2m[G][:, g2 * 128:(g2 + 1) * 128],
                                     h1g[G][:, cs], start=True, stop=True)
                    eng = nc.scalar if h == 0 else nc.vector
                    if G == 0:
                        tgt = h2preS[1] if g2 == 1 else h2preF[g2]
                        if h == 0:
                            eng.activation(tgt[:, cs], pp[:, :], AF.Identity,
                                           bias=b2g[:, g2:g2 + 1])
                        else:
                            eng.tensor_scalar(tgt[:, cs], pp[:, :],
                                              b2g[:, g2:g2 + 1], 0.0,
                                              OP.add, OP.add)
                    else:
                        nc.vector.tensor_tensor(h2preF[g2][:, cs],
                                                h2preF[g2][:, cs],
                                                pp[:, :], OP.add)
                        if g2 == G + 1:
                            if h == 0:
                                nc.scalar.activation(h2preS[g2][:, cs],
                                                     h2preF[g2][:, cs],
                                                     AF.Identity)
                            else:
                                nc.vector.tensor_scalar(
                                    h2preS[g2][:, cs], h2preF[g2][:, cs],
                                    0.0, 0.0, OP.add, OP.add)

        # ---- step 0: bias-only extract, then z_0 into zTbx ----
        alloc_hx(1)
        alloc_hx(2)
        emit_cstage(1)
        emit_cstage(2)
        back_extract(0, 0)
        back_transpose(0, 0)
        back_copy(0, 0)
        back_extract(1, 0)
        back_transpose(1, 0)
        back_copy(1, 0)

        # seg(w, i): front of wave w step i + back of the other wave's
        # pending step (w=0 carries back(1, i-1); w=1 carries back(0, i)).
        def seg(w, i):
            wo = 1 - w
            ib = i - 1 if w == 0 else i
            front1(w, i)
            front2(w, i)
            if ib >= 1:
                back_l3(wo, ib)
            front_relu1(w, i)
            front_h1g(w, i)
            if ib >= 1:
                back_extract(wo, ib)
            front3(w, i)
            if ib >= 1:
                back_transpose(wo, ib)
            front_relu2(w, i)
            if ib >= 1:
                back_copy(wo, ib)

        next_h2 = 18  # first step consuming a DMA-staged h2 prefix
        for i in range(1, D):
            emit_pushes(i)
            seg(0, i)
            seg(1, i)
            alloc_hx(i + 2)
            emit_cstage(i + 2)
            # stage only after the target group's final push was emitted
            while (next_h2 < D and next_h2 <= i + 2
                   and i >= 16 * (next_h2 // 16)):
                if next_h2 in L2SEL_STEPS:
                    next_h2 += 1
                    continue
                emit_h2stage(next_h2)
                next_h2 += 1

        # remaining backs: wave1 step 63 (L3 + scales only)
        back_l3(1, D - 1)
        back_extract(1, D - 1)

        # scales final -> ship first
        for h in range(2):
            nc.sync.dma_start(
                so_d[h * NH:(h + 1) * NH, :].rearrange("(c p) d -> p c d", c=4),
                scB[h][:, :].rearrange("p (c d) -> p c d", c=4))

        # ---- bulk extraction of mu and z ----
        for h in range(2):
            mu_src = outp[h][:, :].rearrange("p (c o) -> p c o", c=4)[:, :, 0:D]
            mu_dst = muB[h][:, :].rearrange("p (c d) -> p c d", c=4)[:, :, :]
            nc.scalar.activation(mu_dst, mu_src, AF.Identity)
            nc.vector.tensor_tensor(zt2[h][:, :], scB[h][:, :], epsB[h][:, :],
                                    OP.mult)
            nc.vector.tensor_tensor(zB[h][:, :], zt2[h][:, :], muB[h][:, :],
                                    OP.add)

        # ---- outputs (batch-major rows r = h*512 + c*128 + p) ----
        for h in range(2):
            dst = slice(h * NH, (h + 1) * NH)
            for name_d, t, oeng in ((zo_d, zB[h], nc.sync),
                                    (mo_d, muB[h], nc.scalar)):
                oeng.dma_start(
                    name_d[dst, :].rearrange("(c p) d -> p c d", c=4),
                    t[:, :].rearrange("p (c d) -> p c d", c=4))

    nc.compile()
    _NC_CACHE["nc"] = nc
    return nc


def kernel(context, eps, W1, b1, Wc, W2, b2, Wo, bo, _trace=False):
    from concourse.bass_utils import run_bass_kernel_spmd

    context = np.asarray(context, np.float32)
    eps = np.asarray(eps, np.float32)
    wd = _host_weights(np.asarray(W1, np.float32), np.asarray(b1, np.float32),
                       np.asarray(Wc, np.float32), np.asarray(W2, np.float32),
                       np.asarray(b2, np.float32), np.asarray(Wo, np.float32),
                       np.asarray(bo, np.float32))

    in_maps = []
    for c in range(NCORES):
        sl = slice(c * BS, (c + 1) * BS)
        ctx_s = context[sl]                       # (1024, 256)
        eps_s = eps[sl]                           # (1024, 64)
        im = dict(wd)
        im["ctxT"] = np.ascontiguousarray(ctx_s.T).astype(BF)
        im["epsB"] = np.ascontiguousarray(
            eps_s.reshape(2, 4, 128, D).transpose(0, 2, 1, 3).reshape(
                2, 128, 4 * D))
        in_maps.append(im)

    nc = _build()
    res = run_bass_kernel_spmd(nc, in_maps, core_ids=list(range(NCORES)),
                               trace=_trace)
    z = np.concatenate([r["zo"] for r in res.results], axis=0)
    mus = np.concatenate([r["mo"] for r in res.results], axis=0)
    scales = np.concatenate([r["so"] for r in res.results], axis=0)
    if _trace:
        kernel.last_exec_time_ns = res.exec_time_ns
        kernel.last_results = res
    return z, mus, scales


# revision 35
# speedup vs baseline: 1.0484x; 1.0484x over previous
"""Trainium2 Bass kernel for MADE autoregressive sampling (rsample).

Structure (degrees mh = arange(512)%63 + 1, sorted into 4 groups of 128):
  - per step i (degree i), the m<=9 units of that degree are computed once;
  - unit-land layout: features on partitions, batch on free dim, column
    j = c*128 + b  <->  batch row (within the 512-col half) c*128 + b
    (chunk-major, c = batch chunk of 128);
  - batch-land: output accumulators outp [128 batch, 4ch x 128 outs].

Critical-path design (vs. the DMA-scatter baseline):
  - z_i is computed batch-land (exp/ln softplus + mult/add on [128, 2]
    chunk-pairs), written into zBb columns, then PE-TRANSPOSED (32-dim
    block x 4 chunks) into PSUM and copied back to the unit-land zTbx
    rows with 32-aligned partition bases -- no per-step DMA round trip
    on the recurrence;
  - the h2 cross-group prefix is read directly from h2preS[g][0:r0+m]
    (partition base 0) via a selector-identity matmul (L2s) accumulating
    into the same PSUM group as L2p/L2d -- no h2-stage DMA;
  - the two batch halves (waves) are software-pipelined: each emission
    segment holds wave w's front (L1/relus/L2/L3) interleaved with the
    other wave's back (extract/transpose/copy), so cross-engine round
    trips of one wave overlap compute of the other;
  - L1 is split into two 256-col matmuls so each copy lane (chunks 0,1 /
    2,3) unblocks its relu1 half independently;
  - ctx projection still uses the staged identity-rows trick in w1x
    (rows 64..81 of zTbx, gpsimd DMA, 2-step lookahead, off-path).
"""

import ml_dtypes
import numpy as np

B, D, CTX, H = 8192, 64, 256, 512
NCORES = 8
BS = B // NCORES   # 1024 rows per core
NH = BS // 2       # 512 per half/wave
MMAX = 9           # max units per degree
KZ = D + 2 * MMAX  # 82: z rows + two ctx-stage slots

BF = ml_dtypes.bfloat16


def _structure():
    m0 = np.arange(1, D + 1)
    mh = (np.arange(H) % (D - 1)) + 1
    M1 = (mh[:, None] >= m0[None, :]).astype(np.float32)   # (H, D)
    M2 = (mh[:, None] >= mh[None, :]).astype(np.float32)   # (H, H)
    mo = np.concatenate([m0, m0])
    Mo = (mo[:, None] > mh[None, :]).astype(np.float32)    # (2D, H)
    perm = np.argsort(mh, kind="stable")
    smh = mh[perm]
    S = np.zeros(D, np.int64)
    E = np.zeros(D, np.int64)
    for i in range(1, D):
        S[i] = np.searchsorted(smh, i, side="left")
        E[i] = np.searchsorted(smh, i, side="right")
    return M1, M2, Mo, perm, S, E


_M1, _M2, _Mo, _PERM, _S, _E = _structure()
assert int(_E[15]) == 128 and int(_E[31]) == 256 and int(_E[47]) == 384

# push schedule: step -> list of (source group G, target group g2).  The
# urgent G->G+1 pushes run at the first step of the target group (the
# boundary step's h1g DMA has landed by then); farther targets spread out.
_PUSHES = {16: [(0, 1)], 18: [(0, 2)], 20: [(0, 3)],
           32: [(1, 2)], 34: [(1, 3)], 48: [(2, 3)]}
# steps whose h2 prefix is read via selector matmul (the group's push
# lands the same step, too late for the 2-step-lookahead stage DMA)
L2SEL_STEPS = {16, 17, 32, 33, 48, 49}


def _host_weights(W1, b1, Wc, W2, b2, Wo, bo):
    W1m = (W1 * _M1).T[:, _PERM]                     # (64, 512)
    W2m = ((W2 * _M2).T)[_PERM][:, _PERM]            # (512, 512)
    Wom = ((Wo * _Mo).T)[_PERM, :]                   # (512, 128)
    Wcs = Wc[_PERM]                                  # (512, 256)
    b1s_ = b1[_PERM]
    b2s_ = b2[_PERM]

    # stacked L1 weights [W1m slice; I@slot(i%2)] and L2 diag+stage
    # [W2 diag; I] (identity rows add the staged h2 cross-group prefix
    # during the same PE pass)
    w1x = np.zeros((KZ, D * MMAX), np.float32)
    w2dx = np.zeros((2 * MMAX, D * MMAX), np.float32)
    selx = np.zeros((128, D * MMAX), np.float32)
    for i in sorted(L2SEL_STEPS):
        s, e = int(_S[i]), int(_E[i])
        r0, m = s % 128, e - s
        for k in range(m):
            selx[r0 + k, i * MMAX + k] = 1.0
    womp = np.zeros((MMAX, D * 2 * D), np.float32)
    b2p = np.zeros((MMAX, D), np.float32)
    for i in range(1, D):
        s, e = int(_S[i]), int(_E[i])
        m = e - s
        w1x[0:D, i * MMAX:i * MMAX + m] = W1m[:, s:e]
        slot = D + MMAX * (i % 2)
        for k in range(m):
            w1x[slot + k, i * MMAX + k] = 1.0
        w2dx[:m, i * MMAX:i * MMAX + m] = W2m[s:e, s:e]
        if i >= 16 and i not in L2SEL_STEPS:
            for k in range(m):
                w2dx[m + k, i * MMAX + k] = 1.0
        womp[:m, i * 2 * D:(i + 1) * 2 * D] = Wom[s:e, :]
        b2p[:m, i] = b2s_[s:e]

    return {
        "w1x": w1x.astype(BF),
        "w2m": np.ascontiguousarray(W2m).astype(BF),
        "wct": np.ascontiguousarray(Wcs.T).astype(BF),        # (256, 512)
        "w2dx": w2dx.astype(BF),
        "selx": selx.astype(BF),
        "womp": womp.astype(BF),
        "b1s": np.ascontiguousarray(b1s_.reshape(4, 128).T, np.float32),
        "b2p": b2p,
        "b2g": np.ascontiguousarray(b2s_.reshape(4, 128).T, np.float32),
        "bo4": np.ascontiguousarray(np.tile(bo, 4)[None, :]).astype(BF),
        "ones": np.ones((1, 128), np.float32).astype(BF),
        "ident": np.eye(128, dtype=np.float32).astype(BF),
    }


_NC_CACHE = {}


def _build():
    if "nc" in _NC_CACHE:
        return _NC_CACHE["nc"]
    from contextlib import ExitStack

    import concourse.mybir as mybir
    import concourse.tile as tile
    from concourse import bacc

    f32 = mybir.dt.float32
    bf16 = mybir.dt.bfloat16
    AF = mybir.ActivationFunctionType
    OP = mybir.AluOpType

    # Only Exp/Ln/Relu/Identity are used -- pin the one ACT table so the
    # greedy selection pass never inserts mid-kernel table loads.
    import concourse.bacc as bacc_mod
    _orig_tables = bacc_mod.get_activation_tables

    def _one_table(arch):
        tabs = _orig_tables(arch)
        return {k: (v if k == "natural_log_exp_and_others" else set())
                for k, v in tabs.items()}

    bacc_mod.get_activation_tables = _one_table

    nc = bacc.Bacc("TRN2", target_bir_lowering=False)

    ctxT_d = nc.dram_tensor("ctxT", [CTX, BS], bf16, kind="ExternalInput")
    epsB_d = nc.dram_tensor("epsB", [2, 128, 4 * D], f32, kind="ExternalInput")
    w1x_d = nc.dram_tensor("w1x", [KZ, D * MMAX], bf16, kind="ExternalInput")
    w2m_d = nc.dram_tensor("w2m", [H, H], bf16, kind="ExternalInput")
    wct_d = nc.dram_tensor("wct", [CTX, H], bf16, kind="ExternalInput")
    w2dx_d = nc.dram_tensor("w2dx", [2 * MMAX, D * MMAX], bf16,
                            kind="ExternalInput")
    selx_d = nc.dram_tensor("selx", [128, D * MMAX], bf16,
                            kind="ExternalInput")
    womp_d = nc.dram_tensor("womp", [MMAX, D * 2 * D], bf16,
                            kind="ExternalInput")
    b1s_d = nc.dram_tensor("b1s", [128, 4], f32, kind="ExternalInput")
    b2p_d = nc.dram_tensor("b2p", [MMAX, D], f32, kind="ExternalInput")
    b2g_d = nc.dram_tensor("b2g", [128, 4], f32, kind="ExternalInput")
    bo4_d = nc.dram_tensor("bo4", [1, 4 * 2 * D], bf16, kind="ExternalInput")
    ones_d = nc.dram_tensor("ones", [1, 128], bf16, kind="ExternalInput")
    ident_d = nc.dram_tensor("ident", [128, 128], bf16, kind="ExternalInput")

    # outputs, batch-major (BS, D); rows r = h*512 + c*128 + p
    zo_d = nc.dram_tensor("zo", [BS, D], f32, kind="ExternalOutput")
    mo_d = nc.dram_tensor("mo", [BS, D], f32, kind="ExternalOutput")
    so_d = nc.dram_tensor("so", [BS, D], f32, kind="ExternalOutput")

    NQ = NH // 2  # 256: column half of a wave (chunk pair)

    with tile.TileContext(nc) as tc, ExitStack() as ctx:
        const = ctx.enter_context(tc.tile_pool(name="const", bufs=1))
        work = ctx.enter_context(tc.tile_pool(name="work", bufs=6))
        pout = ctx.enter_context(tc.tile_pool(name="pout", bufs=1, space="PSUM"))
        pscr = ctx.enter_context(tc.tile_pool(name="pscr", bufs=1, space="PSUM"))

        # ---- constant loads (spread across the 3 DMA queues) ----
        wct = [const.tile([128, H], bf16, name=f"wct{k}") for k in range(2)]
        ctxT = [const.tile([128, BS], bf16, name=f"ctxTs{k}") for k in range(2)]
        for k in range(2):
            nc.sync.dma_start(wct[k][:, :], wct_d[k * 128:(k + 1) * 128, :])
            nc.scalar.dma_start(ctxT[k][:, :],
                                ctxT_d[k * 128:(k + 1) * 128, :])
        w1x = const.tile([KZ, D * MMAX], bf16)
        nc.sync.dma_start(w1x[:, :], w1x_d[:, :])
        w2dx = const.tile([2 * MMAX, D * MMAX], bf16)
        nc.scalar.dma_start(w2dx[:, :], w2dx_d[:, :])
        selx = const.tile([128, D * MMAX], bf16)
        nc.sync.dma_start(selx[:, :], selx_d[:, :])
        womp = const.tile([MMAX, D * 2 * D], bf16)
        nc.scalar.dma_start(womp[:, :], womp_d[:, :])
        b1s = const.tile([128, 4], f32)
        nc.sync.dma_start(b1s[:, :], b1s_d[:, :])
        b2p = const.tile([MMAX, D], f32)
        nc.scalar.dma_start(b2p[:, :], b2p_d[:, :])
        b2g = const.tile([128, 4], f32)
        nc.sync.dma_start(b2g[:, :], b2g_d[:, :])
        bo4 = const.tile([1, 4 * 2 * D], bf16)
        nc.sync.dma_start(bo4[:, :], bo4_d[:, :])
        ones = const.tile([1, 128], bf16)
        nc.scalar.dma_start(ones[:, :], ones_d[:, :])
        ident = const.tile([128, 128], bf16)
        nc.sync.dma_start(ident[:, :], ident_d[:, :])
        epsB = [const.tile([128, 4 * D], f32, name=f"epsB{h}") for h in range(2)]
        for h in range(2):
            nc.scalar.dma_start(epsB[h][:, :], epsB_d[h, :, :])
        w2m = [const.tile([128, H], bf16, name=f"w2m{g}") for g in range(4)]
        for g in range(4):
            (nc.sync if g % 2 else nc.scalar).dma_start(
                w2m[g][:, :], w2m_d[g * 128:(g + 1) * 128, :])

        # ---- state tiles ----
        zTbx = const.tile([KZ, BS], bf16)      # unit-land z + ctx slots
        nc.vector.memset(zTbx[:, :], 0.0)
        zBb = [const.tile([128, 4 * D], bf16, name=f"zBb{h}") for h in range(2)]
        for h in range(2):
            nc.gpsimd.memset(zBb[h][:, :], 0.0)
        h1g = [const.tile([128, BS], bf16, name=f"h1g{g}") for g in range(4)]
        cbg = [const.tile([128, BS], bf16, name=f"cbg{g}") for g in range(4)]
        h2preS = {g: const.tile([128, BS], bf16, name=f"h2preS{g}")
                  for g in (1, 2, 3)}
        h2preF = {g: const.tile([128, BS], f32, name=f"h2preF{g}")
                  for g in (2, 3)}
        muB = [const.tile([128, 4 * D], f32, name=f"muB{h}") for h in range(2)]
        scB = [const.tile([128, 4 * D], f32, name=f"scB{h}") for h in range(2)]
        zB = [const.tile([128, 4 * D], f32, name=f"zB{h}") for h in range(2)]
        zt2 = [const.tile([128, 4 * D], f32, name=f"zt2{h}") for h in range(2)]

        # persistent PSUM: output accumulators + transposed-z staging
        outp = [pout.tile([128, 4 * 128], f32, name=f"outp{h}", tag=f"op{h}")
                for h in range(2)]
        zT = pout.tile([D, BS], bf16, name="zT", tag="zT")
        fil = pout.tile([1, NH], f32, name="fil", tag="fil")

        def ov(h, ch):            # (128, 128) chunk view of the accumulator
            return outp[h][:, ch * 128:(ch + 1) * 128]

        def ocol(h, o):           # (128, 4) strided column view, output o
            return outp[h][:, :].rearrange("p (c o) -> p c o", c=4)[:, :, o]

        def bcol(t, i):           # (128, 4)
            return t[:, :].rearrange("p (c d) -> p c d", c=4)[:, :, i]

        # ---- init: bias rows (one f=512 matmul per half) ----
        for h in range(2):
            nc.tensor.matmul(outp[h][:, :], ones[0:1, :], bo4[0:1, :],
                             start=True, stop=True)

        # ---- init: ctx projection (+b1), unit-land, bf16 out ----
        for g in range(4):
            for h in range(2):
                cs = slice(h * NH, (h + 1) * NH)
                pc = pscr.tile([128, NH], f32, tag="pp", bufs=2,
                               name=f"pc{g}_{h}")
                for k in range(2):
                    nc.tensor.matmul(pc[:, :], wct[k][:, g * 128:(g + 1) * 128],
                                     ctxT[k][:, cs], start=(k == 0),
                                     stop=(k == 1))
                eng = nc.scalar if (g + h) % 2 == 0 else nc.vector
                if eng is nc.scalar:
                    eng.activation(cbg[g][:, cs], pc[:, :], AF.Identity,
                                   bias=b1s[:, g:g + 1])
                else:
                    eng.tensor_scalar(cbg[g][:, cs], pc[:, :], b1s[:, g:g + 1],
                                      0.0, OP.add, OP.add)

        # ---- stage helper: ctx rows for step i into zTbx slot i%2 ----
        def emit_cstage(i):
            if i >= D:
                return
            s, e = int(_S[i]), int(_E[i])
            g, r0, m = s // 128, s % 128, e - s
            slot = D + MMAX * (i % 2)
            nc.gpsimd.dma_start(zTbx[slot:slot + m, :], cbg[g][r0:r0 + m, :])

        hx = {}

        def alloc_hx(i):
            if i < D:
                hx[i] = work.tile([2 * MMAX, BS], bf16, tag="hx",
                                  name=f"hx{i}")

        def emit_h2stage(i):
            s, e = int(_S[i]), int(_E[i])
            g, r0, m = s // 128, s % 128, e - s
            nc.gpsimd.dma_start(hx[i][m:2 * m, :], h2preS[g][r0:r0 + m, :])

        # p1 lives at partitions 0:9 and p2 at 32:41 of one shared PSUM
        # bank per wave (32-aligned bases are legal compute APs).
        p12t = {}

        def front1(w, i):
            """L1 into p1 (rows 0:9 of the shared bank), 256-col lanes."""
            s, e = int(_S[i]), int(_E[i])
            m = e - s
            p12 = pscr.tile([32 + MMAX, NH], f32, tag=f"p12_{w}",
                            name=f"p12_{i}_{w}")
            p12t[w] = p12
            for q in range(2):
                csq = slice(w * NH + q * NQ, w * NH + (q + 1) * NQ)
                nc.tensor.matmul(p12[0:m, q * NQ:(q + 1) * NQ],
                                 w1x[:, i * MMAX:i * MMAX + m],
                                 zTbx[:, csq], start=True, stop=True)

        def front2(w, i):
            """L2 prefix matmul (within-group, independent of z_i)."""
            s, e = int(_S[i]), int(_E[i])
            g, r0, m = s // 128, s % 128, e - s
            if r0 > 0:
                cs = slice(w * NH, (w + 1) * NH)
                nc.tensor.matmul(p12t[w][32:32 + m, :], w2m[g][0:r0, s:e],
                                 h1g[g][0:r0, cs], start=True, stop=False)

        def front_relu1(w, i):
            s, e = int(_S[i]), int(_E[i])
            m = e - s
            p1 = p12t[w]
            ca = slice(w * NH, w * NH + NQ)
            cb = slice(w * NH + NQ, (w + 1) * NH)
            nc.scalar.activation(hx[i][0:m, ca], p1[0:m, 0:NQ], AF.Relu)
            nc.vector.tensor_relu(hx[i][0:m, cb], p1[0:m, NQ:NH])

        def front_h1g(w, i):
            if i >= D - 1:
                return    # no consumer for the last step's h1
            s, e = int(_S[i]), int(_E[i])
            g, r0, m = s // 128, s % 128, e - s
            cs = slice(w * NH, (w + 1) * NH)
            nc.sync.dma_start(h1g[g][r0:r0 + m, cs], hx[i][0:m, cs])

        def front3(w, i):
            """L2s (h2 cross-group prefix via selector) + L2d (diag)."""
            s, e = int(_S[i]), int(_E[i])
            g, r0, m = s // 128, s % 128, e - s
            p2 = p12t[w][32:32 + MMAX, :]
            if i in L2SEL_STEPS:
                k2 = m
            else:
                k2 = 2 * m if g >= 1 else m
            for q in range(2):
                csq = slice(w * NH + q * NQ, w * NH + (q + 1) * NQ)
                qs = slice(q * NQ, (q + 1) * NQ)
                started = r0 > 0
                if i in L2SEL_STEPS:
                    nc.tensor.matmul(p2[0:m, qs],
                                     selx[0:r0 + m, i * MMAX:i * MMAX + m],
                                     h2preS[g][0:r0 + m, csq],
                                     start=not started, stop=False)
                    started = True
                nc.tensor.matmul(p2[0:m, qs],
                                 w2dx[0:k2, i * MMAX:i * MMAX + m],
                                 hx[i][0:k2, csq], start=not started, stop=True)

        h2nt = {}

        def front_relu2(w, i):
            s, e = int(_S[i]), int(_E[i])
            g, m = s // 128, e - s
            p2 = p12t[w][32:32 + MMAX, :]
            t = work.tile([MMAX, NH], bf16, tag=f"h2n{w}", name=f"h2n_{i}_{w}")
            h2nt[w] = t
            if g == 0:
                nc.scalar.activation(t[0:m, 0:NQ], p2[0:m, 0:NQ], AF.Relu,
                                     bias=b2p[0:m, i:i + 1])
                nc.vector.tensor_scalar(t[0:m, NQ:NH], p2[0:m, NQ:NH],
                                        b2p[0:m, i:i + 1], 0.0,
                                        OP.add, OP.max)
            else:
                nc.scalar.activation(t[0:m, 0:NQ], p2[0:m, 0:NQ], AF.Relu)
                nc.vector.tensor_relu(t[0:m, NQ:NH], p2[0:m, NQ:NH])

        def back_l3(w, i):
            s, e = int(_S[i]), int(_E[i])
            m = e - s
            t = h2nt[w]
            for ch in range(4):
                nc.tensor.matmul(ov(w, ch), t[0:m, ch * 128:(ch + 1) * 128],
                                 womp[0:m, i * 2 * D:(i + 1) * 2 * D],
                                 start=False, stop=True)

        def back_extract(w, i):
            """softplus + z_i, batch-land [128, 4]."""
            nc.scalar.activation(bcol(scB[w], i), ocol(w, D + i), AF.Exp)
            nc.scalar.activation(bcol(scB[w], i), bcol(scB[w], i),
                                 AF.Ln, bias=1.0)
            if i >= D - 1:
                return
            zt = work.tile([128, 4], f32, tag=f"zt{w}", name=f"zt{i}_{w}")
            nc.vector.tensor_tensor(zt[:, :], bcol(scB[w], i),
                                    bcol(epsB[w], i), OP.mult)
            nc.vector.tensor_tensor(bcol(zBb[w], i), zt[:, :],
                                    ocol(w, i), OP.add)

        def back_transpose(w, i):
            if i >= D - 1:
                return
            b32 = (i // 32) * 32
            for c in range(4):
                nc.tensor.transpose(
                    zT[b32:b32 + 32, w * NH + c * 128:w * NH + (c + 1) * 128],
                    zBb[w][:, c * D + b32:c * D + b32 + 32],
                    ident[:, :])

        def back_copy(w, i):
            if i >= D - 1:
                return
            b32 = (i // 32) * 32
            cs0 = slice(w * NH, w * NH + NQ)
            cs1 = slice(w * NH + NQ, (w + 1) * NH)
            nc.scalar.activation(zTbx[b32:b32 + 32, cs0],
                                 zT[b32:b32 + 32, cs0], AF.Identity)
            nc.vector.tensor_scalar(zTbx[b32:b32 + 32, cs1],
                                    zT[b32:b32 + 32, cs1], 0.0, 0.0,
                                    OP.add, OP.add)

        def emit_pushes(i):
            for G, g2 in _PUSHES.get(i, ()):
                for h in range(2):
                    cs = slice(h * NH, (h + 1) * NH)
                    pp = pscr.tile([128, NH], f32, tag="pp", bufs=2,
                                   name=f"push{G}_{g2}_{h}")
                    nc.tensor.matmul(pp[:, :],
                                     w2m[G][:, g2 * 128:(g2 + 1) * 128],
                                     h1g[G][:, cs], start=True, stop=True)
                    eng = nc.scalar if h == 0 else nc.vector
                    if G == 0:
                        tgt = h2preS[1] if g2 == 1 else h2preF[g2]
                        if h == 0:
                            eng.activation(tgt[:, cs], pp[:, :], AF.Identity,
                                           bias=b2g[:, g2:g2 + 1])
                        else:
                            eng.tensor_scalar(tgt[:, cs], pp[:, :],
                                              b2g[:, g2:g2 + 1], 0.0,
                                              OP.add, OP.add)
                    else:
                        nc.vector.tensor_tensor(h2preF[g2][:, cs],
                                                h2preF[g2][:, cs],
                                                pp[:, :], OP.add)
                        if g2 == G + 1:
                            if h == 0:
                                nc.scalar.activation(h2preS[g2][:, cs],
                                                     h2preF[g2][:, cs],
                                                     AF.Identity)
                            else:
                                nc.vector.tensor_scalar(
                                    h2preS[g2][:, cs], h2preF[g2][:, cs],
                                    0.0, 0.0, OP.add, OP.add)

        # ---- step 0: bias-only extract, then z_0 into zTbx ----
        alloc_hx(1)
        alloc_hx(2)
        emit_cstage(1)
        emit_cstage(2)
        back_extract(0, 0)
        back_transpose(0, 0)
        back_copy(0, 0)
        back_extract(1, 0)
        back_transpose(1, 0)
        back_copy(1, 0)

        # seg(w, i): front of wave w step i + back of the other wave's
        # pending step (w=0 carries back(1, i-1); w=1 carries back(0, i)).
        def seg(w, i):
            wo = 1 - w
            ib = i - 1 if w == 0 else i
            front1(w, i)
            front2(w, i)
            if ib >= 1:
                back_l3(wo, ib)
            front_relu1(w, i)
            front_h1g(w, i)
            if ib >= 1:
                back_extract(wo, ib)
            front3(w, i)
            if ib >= 1:
                back_transpose(wo, ib)
            front_relu2(w, i)
            if ib >= 1:
                back_copy(wo, ib)

        next_h2 = 18  # first step consuming a DMA-staged h2 prefix
        for i in range(1, D):
            emit_pushes(i)
            seg(0, i)
            seg(1, i)
            alloc_hx(i + 2)
            emit_cstage(i + 2)
            # stage only after the target group's final push was emitted
            while (next_h2 < D and next_h2 <= i + 2
                   and i >= 16 * (next_h2 // 16)):
                if next_h2 in L2SEL_STEPS:
                    next_h2 += 1
                    continue
                emit_h2stage(next_h2)
                next_h2 += 1

        # remaining backs: wave1 step 63 (L3 + scales only)
        back_l3(1, D - 1)
        back_extract(1, D - 1)

        # scales final -> ship first
        for h in range(2):
            nc.sync.dma_start(
                so_d[h * NH:(h + 1) * NH, :].rearrange("(c p) d -> p c d", c=4),
                scB[h][:, :].rearrange("p (c d) -> p c d", c=4))

        # ---- bulk extraction of mu and z ----
        for h in range(2):
            mu_src = outp[h][:, :].rearrange("p (c o) -> p c o", c=4)[:, :, 0:D]
            mu_dst = muB[h][:, :].rearrange("p (c d) -> p c d", c=4)[:, :, :]
            nc.scalar.activation(mu_dst, mu_src, AF.Identity)
            nc.vector.tensor_tensor(zt2[h][:, :], scB[h][:, :], epsB[h][:, :],
                                    OP.mult)
            nc.vector.tensor_tensor(zB[h][:, :], zt2[h][:, :], muB[h][:, :],
                                    OP.add)

        # ---- outputs (batch-major rows r = h*512 + c*128 + p) ----
        for h in range(2):
            dst = slice(h * NH, (h + 1) * NH)
            for name_d, t, oeng in ((zo_d, zB[h], nc.sync),
                                    (mo_d, muB[h], nc.scalar)):
                oeng.dma_start(
                    name_d[dst, :].rearrange("(c p) d -> p c d", c=4),
                    t[:, :].rearrange("p (c d) -> p c d", c=4))

    nc.compile()
    _NC_CACHE["nc"] = nc
    return nc


def kernel(context, eps, W1, b1, Wc, W2, b2, Wo, bo, _trace=False):
    from concourse.bass_utils import run_bass_kernel_spmd

    context = np.asarray(context, np.float32)
    eps = np.asarray(eps, np.float32)
    wd = _host_weights(np.asarray(W1, np.float32), np.asarray(b1, np.float32),
                       np.asarray(Wc, np.float32), np.asarray(W2, np.float32),
                       np.asarray(b2, np.float32), np.asarray(Wo, np.float32),
                       np.asarray(bo, np.float32))

    in_maps = []
    for c in range(NCORES):
        sl = slice(c * BS, (c + 1) * BS)
        ctx_s = context[sl]                       # (1024, 256)
        eps_s = eps[sl]                           # (1024, 64)
        im = dict(wd)
        im["ctxT"] = np.ascontiguousarray(ctx_s.T).astype(BF)
        im["epsB"] = np.ascontiguousarray(
            eps_s.reshape(2, 4, 128, D).transpose(0, 2, 1, 3).reshape(
                2, 128, 4 * D))
        in_maps.append(im)

    nc = _build()
    res = run_bass_kernel_spmd(nc, in_maps, core_ids=list(range(NCORES)),
                               trace=_trace)
    z = np.concatenate([r["zo"] for r in res.results], axis=0)
    mus = np.concatenate([r["mo"] for r in res.results], axis=0)
    scales = np.concatenate([r["so"] for r in res.results], axis=0)
    if _trace:
        kernel.last_exec_time_ns = res.exec_time_ns
        kernel.last_results = res
    return z, mus, scales
